# revision 3
# baseline (speedup 1.0000x reference)
"""RWKV7 block kernel for 8 trn2 NeuronCores.

Strategy v1: all six large matmuls (W_r/W_k/W_v/W_o, cm_key, cm_val -- ~95% of
FLOPs) run on the 8 NeuronCores through a token-sharded bass/Tile matmul
(rows of the activation split 1024/core, weights replicated, bf16 inputs with
fp32 PSUM accumulation). The remaining pointwise/LN/small-matmul glue and the
WKV7 scan (jax.lax.scan on CPU, identical math to the reference) run on host.
"""
import numpy as np
import ml_dtypes

import concourse.tile as tile
from concourse import bacc, mybir
from concourse.bass_utils import run_bass_kernel_spmd

B, T, C = 4, 2048, 2048
H, HS, FFN = 32, 64, 8192
NCORES = 8
ROWS = B * T // NCORES  # 1024 rows per core

_CACHE = {}


def _build_mm(K, N):
    KT = K // 128
    KS = min(KT, 16)  # k-tiles per resident slab
    nc = bacc.Bacc("TRN2", target_bir_lowering=False, debug=False,
                   enable_asserts=True, num_devices=NCORES)
    at = nc.dram_tensor("at", [K, ROWS], mybir.dt.bfloat16, kind="ExternalInput")
    bm = nc.dram_tensor("bm", [K, N], mybir.dt.bfloat16, kind="ExternalInput")
    out = nc.dram_tensor("out", [ROWS, N], mybir.dt.float32, kind="ExternalOutput")
    atr = at.ap().rearrange("(ko ks p) m -> ko p ks m", p=128, ks=KS)
    with tile.TileContext(nc) as tc:
        with tc.tile_pool(name="ap", bufs=2) as apool, \
             tc.tile_pool(name="bp", bufs=2) as bpool, \
             tc.tile_pool(name="op", bufs=4) as opool, \
             tc.tile_pool(name="ps", bufs=1, space="PSUM") as ps:
            for n in range(N // 512):
                pts = [ps.tile([128, 512], mybir.dt.float32, name=f"pt{i}", tag=f"pt{i}")
                       for i in range(8)]
                for ko in range(KT // KS):
                    a_sb = apool.tile([128, KS, ROWS], mybir.dt.bfloat16)
                    nc.sync.dma_start(a_sb[:], atr[ko])
                    b_sb = bpool.tile([128, KS, 512], mybir.dt.bfloat16)
                    nc.sync.dma_start(
                        b_sb[:],
                        bm.ap()[:, n * 512:(n + 1) * 512]
                        .rearrange("(ko ks p) f -> ko p ks f", p=128, ks=KS)[ko])
                    for m in range(8):
                        for k in range(KS):
                            nc.tensor.matmul(
                                pts[m][:],
                                a_sb[:, k, m * 128:(m + 1) * 128],
                                b_sb[:, k, :],
                                start=(ko == 0 and k == 0),
                                stop=(ko == KT // KS - 1 and k == KS - 1))
                for m in range(8):
                    ot = opool.tile([128, 512], mybir.dt.float32)
                    nc.any.tensor_copy(ot[:], pts[m][:])
                    nc.sync.dma_start(
                        out.ap()[m * 128:(m + 1) * 128, n * 512:(n + 1) * 512], ot[:])
    nc.compile()
    return nc


def dev_mm(A, Bm):
    """A [8192, K] @ Bm [K, N] -> [8192, N] f32, bf16 compute on 8 cores."""
    K, N = Bm.shape
    key = (K, N)
    if key not in _CACHE:
        _CACHE[key] = _build_mm(K, N)
    nc = _CACHE[key]
    abf = np.ascontiguousarray(A.T.astype(ml_dtypes.bfloat16))  # [K, 8192]
    bbf = np.ascontiguousarray(Bm.astype(ml_dtypes.bfloat16))
    ins = [{"at": np.ascontiguousarray(abf[:, i * ROWS:(i + 1) * ROWS]),
            "bm": bbf} for i in range(NCORES)]
    res = run_bass_kernel_spmd(nc, ins, core_ids=list(range(NCORES)))
    return np.concatenate([r["out"] for r in res.results], 0)


def _layernorm(y):
    mu = y.mean(-1, keepdims=True)
    var = y.var(-1, keepdims=True)
    return (y - mu) / np.sqrt(var + 1e-5)


def _time_shift(y):
    out = np.zeros_like(y)
    out[:, 1:] = y[:, :-1]
    return out


def _sig(z):
    return 1.0 / (1.0 + np.exp(-z))


_WKV = {}


def _wkv7(r, w, k, v, a, b):
    import os
    os.environ.setdefault('JAX_PLATFORMS', 'axon,cpu')
    import jax
    import jax.numpy as jnp

    if 'fn' not in _WKV:
        def wkv7_jax(r, w, k, v, a, b):
            Bz, Tt, Cc = r.shape
            def bh(t):
                return t.astype(jnp.bfloat16).astype(jnp.float32).reshape(Bz, Tt, H, HS)
            r, k, v, a, b = map(bh, (r, k, v, a, b))
            wdec = jnp.exp(-jnp.exp(w.astype(jnp.bfloat16).astype(jnp.float32))).reshape(Bz, Tt, H, HS)
            def step(S, inp):
                rt, wt, kt, vt, at, bt = inp
                Sa = jnp.einsum('bhij,bhj->bhi', S, at)
                S = S * wt[:, :, None, :] + Sa[..., None] * bt[:, :, None, :] + vt[..., None] * kt[:, :, None, :]
                return S, jnp.einsum('bhij,bhj->bhi', S, rt)
            S0 = jnp.zeros((Bz, H, HS, HS), jnp.float32)
            seq = tuple(jnp.moveaxis(t, 1, 0) for t in (r, wdec, k, v, a, b))
            _, out = jax.lax.scan(step, S0, seq)
            return jnp.moveaxis(out, 0, 1).reshape(Bz, Tt, Cc)
        cpu = jax.devices('cpu')[0]
        _WKV['fn'] = jax.jit(wkv7_jax, device=cpu)
        _WKV['cpu'] = cpu
    with jax.default_device(_WKV['cpu']):
        out = _WKV['fn'](r, w, k, v, a, b)
    return np.asarray(out)


def kernel(x, params):
    x = np.asarray(x, np.float32)
    p = {k: np.asarray(v, np.float32) for k, v in params.items()}

    xf = x.reshape(B * T, C)

    # ---- tmix ----
    xln3 = _layernorm(x)
    xx = (_time_shift(xln3) - xln3)
    xxx = xln3 + xx * p['maa_x']
    m = np.tanh((xxx.reshape(B * T, C) @ p['maa_w1'])).reshape(B, T, 4, 32)
    mrg = (m[:, :, 0].reshape(B * T, 32) @ p['maa_w2'][0]).reshape(B, T, C)
    mwa = (m[:, :, 1].reshape(B * T, 32) @ p['maa_w2'][1]).reshape(B, T, C)
    mk_ = (m[:, :, 2].reshape(B * T, 32) @ p['maa_w2'][2]).reshape(B, T, C)
    mv = (m[:, :, 3].reshape(B * T, 32) @ p['maa_w2'][3]).reshape(B, T, C)
    xrg = (xln3 + xx * (p['maa_rg'] + mrg)).reshape(B * T, C)
    xwa = (xln3 + xx * (p['maa_wa'] + mwa)).reshape(B * T, C)
    xk = (xln3 + xx * (p['maa_k'] + mk_)).reshape(B * T, C)
    xv = (xln3 + xx * (p['maa_v'] + mv)).reshape(B * T, C)

    r = dev_mm(xrg, p['W_r'])
    k = dev_mm(xk, p['W_k'])
    v = dev_mm(xv, p['W_v'])

    w = -np.logaddexp(0.0, -(p['time_decay'] + np.tanh(xwa @ p['dec_w1']) @ p['dec_w2'])) - 0.5
    g = np.tanh(xrg @ p['gate_w1']) @ p['gate_w2']
    kk = k + np.tanh(xk @ p['kkk_w1']) @ p['kkk_w2']
    kkh = kk.reshape(B * T, H, HS)
    kk = (kkh / np.maximum(np.linalg.norm(kkh, axis=-1, keepdims=True), 1e-12)).reshape(B * T, C)
    a = _sig(p['time_aaaaa'] + xwa @ p['aaa_w1'] @ p['aaa_w2'])
    ma = _sig(xwa @ p['ma_w1'] @ p['ma_w2'])
    k = k * ma + k * a * (1.0 - ma)
    mk2 = _sig(xk @ p['mk_w1'] @ p['mk_w2'])
    k = k * np.exp(np.minimum(w * mk2, 0.0))

    sh = (B, T, C)
    o = _wkv7(r.reshape(sh), w.reshape(sh), k.reshape(sh), v.reshape(sh),
              (-kk).reshape(sh), (kk * a).reshape(sh)).reshape(B * T, C)

    og = o.reshape(B * T, H, HS)
    mu = og.mean(-1, keepdims=True)
    var = og.var(-1, keepdims=True)
    o = ((og - mu) / np.sqrt(var + 64e-5)).reshape(B * T, C)
    rk = (r.reshape(B * T, H, HS) * k.reshape(B * T, H, HS) * p['time_faaaa']).sum(-1, keepdims=True)
    o = o + (rk * v.reshape(B * T, H, HS)).reshape(B * T, C)

    h = xf + dev_mm(o * g, p['W_o'])

    # ---- cmix ----
    h3 = h.reshape(B, T, C)
    hln = _layernorm(h3)
    xx2 = _time_shift(hln) - hln
    kf = (hln + xx2 * p['cm_maa_k']).reshape(B * T, C)
    kf = np.square(np.maximum(dev_mm(kf, p['cm_key']), 0.0))
    out = h + dev_mm(kf, p['cm_val'])
    return out.reshape(B, T, C).astype(np.float32)


# revision 15
# speedup vs baseline: 1.1532x; 1.1532x over previous
"""RWKV7 block kernel for 8 trn2 NeuronCores.

Strategy v1: all six large matmuls (W_r/W_k/W_v/W_o, cm_key, cm_val -- ~95% of
FLOPs) run on the 8 NeuronCores through a token-sharded bass/Tile matmul
(rows of the activation split 1024/core, weights replicated, bf16 inputs with
fp32 PSUM accumulation). The remaining pointwise/LN/small-matmul glue and the
WKV7 scan (jax.lax.scan on CPU, identical math to the reference) run on host.
"""
import os

_jp = os.environ.get('JAX_PLATFORMS')
if _jp is None:
    os.environ['JAX_PLATFORMS'] = 'cpu'
elif 'cpu' not in _jp.split(','):
    os.environ['JAX_PLATFORMS'] = _jp + ',cpu'

import numpy as np
import ml_dtypes

import concourse.tile as tile
from concourse import bacc, mybir
from concourse.bass_utils import run_bass_kernel_spmd

B, T, C = 4, 2048, 2048
H, HS, FFN = 32, 64, 8192
NCORES = 8
ROWS = B * T // NCORES  # 1024 rows per core

_CACHE = {}


def _build_mm(K, N):
    KT = K // 128
    KS = min(KT, 16)  # k-tiles per resident slab
    nc = bacc.Bacc("TRN2", target_bir_lowering=False, debug=False,
                   enable_asserts=True, num_devices=NCORES)
    at = nc.dram_tensor("at", [K, ROWS], mybir.dt.bfloat16, kind="ExternalInput")
    bm = nc.dram_tensor("bm", [K, N], mybir.dt.bfloat16, kind="ExternalInput")
    out = nc.dram_tensor("out", [ROWS, N], mybir.dt.float32, kind="ExternalOutput")
    atr = at.ap().rearrange("(ko ks p) m -> ko p ks m", p=128, ks=KS)
    with tile.TileContext(nc) as tc:
        with tc.tile_pool(name="ap", bufs=2) as apool, \
             tc.tile_pool(name="bp", bufs=2) as bpool, \
             tc.tile_pool(name="op", bufs=4) as opool, \
             tc.tile_pool(name="ps", bufs=1, space="PSUM") as ps:
            for n in range(N // 512):
                pts = [ps.tile([128, 512], mybir.dt.float32, name=f"pt{i}", tag=f"pt{i}")
                       for i in range(8)]
                for ko in range(KT // KS):
                    a_sb = apool.tile([128, KS, ROWS], mybir.dt.bfloat16)
                    nc.sync.dma_start(a_sb[:], atr[ko])
                    b_sb = bpool.tile([128, KS, 512], mybir.dt.bfloat16)
                    nc.sync.dma_start(
                        b_sb[:],
                        bm.ap()[:, n * 512:(n + 1) * 512]
                        .rearrange("(ko ks p) f -> ko p ks f", p=128, ks=KS)[ko])
                    for m in range(8):
                        for k in range(KS):
                            nc.tensor.matmul(
                                pts[m][:],
                                a_sb[:, k, m * 128:(m + 1) * 128],
                                b_sb[:, k, :],
                                start=(ko == 0 and k == 0),
                                stop=(ko == KT // KS - 1 and k == KS - 1))
                for m in range(8):
                    ot = opool.tile([128, 512], mybir.dt.float32)
                    nc.any.tensor_copy(ot[:], pts[m][:])
                    nc.sync.dma_start(
                        out.ap()[m * 128:(m + 1) * 128, n * 512:(n + 1) * 512], ot[:])
    nc.compile()
    return nc


_BCACHE = {}
_RUNNERS = {}
_TIMING = os.environ.get('KERNEL_TIMING', '') == '1'


def _make_runner(nc, K, N, in_names_exp=None):
    """Cached shard_map runner mirroring bass2jax.run_bass_via_pjrt, with the
    weight operand kept device-resident across calls."""
    import jax
    from jax.sharding import Mesh, PartitionSpec, NamedSharding
    from jax.experimental.shard_map import shard_map
    from concourse import bass2jax, mybir as _mybir

    bass2jax.install_neuronx_cc_hook()
    part_name = nc.partition_id_tensor.name if nc.partition_id_tensor else None
    in_names, out_names, out_avals, zero_shapes = [], [], [], []
    for alloc in nc.m.functions[0].allocations:
        if not isinstance(alloc, _mybir.MemoryLocationSet):
            continue
        name = alloc.memorylocations[0].name
        if alloc.kind == "ExternalInput":
            if name != part_name:
                in_names.append(name)
        elif alloc.kind == "ExternalOutput":
            shape = tuple(alloc.tensor_shape)
            dtype = _mybir.dt.np(alloc.dtype)
            out_names.append(name)
            out_avals.append(jax.core.ShapedArray(shape, dtype))
            zero_shapes.append((shape, dtype))
    assert in_names == (in_names_exp or ["at", "bm"]) and out_names == ["out"], \
        (in_names, out_names)
    n_params = len(in_names)
    all_in_names = list(in_names + out_names)
    if part_name is not None:
        all_in_names.append(part_name)
    all_in_names = tuple(all_in_names)
    donate = tuple(range(n_params, n_params + len(out_names)))

    def _body(*args):
        operands = list(args)
        if part_name is not None:
            operands.append(bass2jax.partition_id_tensor())
        outs = bass2jax._bass_exec_p.bind(
            *operands,
            out_avals=tuple(out_avals),
            in_names=all_in_names,
            out_names=tuple(out_names),
            lowering_input_output_aliases=(),
            sim_require_finite=True,
            sim_require_nnan=True,
            nc=nc,
        )
        return tuple(outs)

    devices = jax.devices()[:NCORES]
    mesh = Mesh(np.asarray(devices), ("core",))
    spec = PartitionSpec("core")
    in_specs = (spec,) * (n_params + len(out_names))
    out_specs = (spec,) * len(out_names)
    fn = jax.jit(
        shard_map(_body, mesh=mesh, in_specs=in_specs, out_specs=out_specs,
                  check_rep=False),
        donate_argnums=donate, keep_unused=True)
    sharding = NamedSharding(mesh, spec)

    def run(*inputs):
        zeros = np.zeros((NCORES * zero_shapes[0][0][0], *zero_shapes[0][0][1:]),
                         zero_shapes[0][1])
        (out,) = fn(*inputs, zeros)
        return np.asarray(out)

    def put_weight(bbf):
        import jax as _jax
        return _jax.device_put(np.concatenate([bbf] * NCORES, 0), sharding)

    return run, put_weight


def _build_ffn():
    """kf = relu(A @ cm_key)^2; out = kf @ cm_val, per core rows=1024."""
    nc = bacc.Bacc("TRN2", target_bir_lowering=False, debug=False,
                   enable_asserts=True, num_devices=NCORES)
    at = nc.dram_tensor("at", [C, ROWS], mybir.dt.bfloat16, kind="ExternalInput")
    wk = nc.dram_tensor("wk", [C, FFN], mybir.dt.bfloat16, kind="ExternalInput")
    wv = nc.dram_tensor("wv", [FFN, C], mybir.dt.bfloat16, kind="ExternalInput")
    out = nc.dram_tensor("out", [ROWS, C], mybir.dt.float32, kind="ExternalOutput")
    KT = C // 128      # 16 k-tiles over C
    FT = FFN // 128    # 64 ffn tiles
    with tile.TileContext(nc) as tc:
        with tc.tile_pool(name="kfp", bufs=1) as kfp, \
             tc.tile_pool(name="ps", bufs=1, space="PSUM") as ps:
            kfT = kfp.tile([128, FT, ROWS], mybir.dt.bfloat16)  # 128KB/part
            with tc.tile_pool(name="ap", bufs=1) as apool, \
                 tc.tile_pool(name="wkp", bufs=3) as wkp, \
                 tc.tile_pool(name="rl", bufs=4) as rlp:
                a_sb = apool.tile([128, KT, ROWS], mybir.dt.bfloat16)
                nc.sync.dma_start(a_sb[:], at.ap().rearrange("(kt p) m -> p kt m", p=128))
                for f in range(FT):
                    wk_sb = wkp.tile([128, KT, 128], mybir.dt.bfloat16)
                    nc.sync.dma_start(
                        wk_sb[:],
                        wk.ap()[:, f * 128:(f + 1) * 128]
                        .rearrange("(kt p) n -> p kt n", p=128))
                    for n2 in range(ROWS // 512):
                        pt = ps.tile([128, 512], mybir.dt.float32, name="pt_a", tag="pt0")
                        for k in range(KT):
                            nc.tensor.matmul(
                                pt[:], wk_sb[:, k, :],
                                a_sb[:, k, n2 * 512:(n2 + 1) * 512],
                                start=(k == 0), stop=(k == KT - 1))
                        rl = rlp.tile([128, 512], mybir.dt.float32)
                        nc.scalar.activation(rl[:], pt[:],
                                             mybir.ActivationFunctionType.Relu)
                        nc.vector.tensor_mul(
                            kfT[:, f, n2 * 512:(n2 + 1) * 512], rl[:], rl[:])
            with tc.tile_pool(name="wvp", bufs=3) as wvp, \
                 tc.tile_pool(name="op", bufs=4) as opool:
                FS = 16  # ffn k-tiles per streamed cm_val slab
                for n in range(C // 512):
                    pts = [ps.tile([128, 512], mybir.dt.float32,
                                   name=f"pt{i}", tag=f"pt{i}") for i in range(8)]
                    for fo in range(FT // FS):
                        wv_sb = wvp.tile([128, FS, 512], mybir.dt.bfloat16)
                        nc.sync.dma_start(
                            wv_sb[:],
                            wv.ap()[:, n * 512:(n + 1) * 512]
                            .rearrange("(fo fs p) c -> fo p fs c", p=128, fs=FS)[fo])
                        for m in range(8):
                            for k in range(FS):
                                nc.tensor.matmul(
                                    pts[m][:],
                                    kfT[:, fo * FS + k, m * 128:(m + 1) * 128],
                                    wv_sb[:, k, :],
                                    start=(fo == 0 and k == 0),
                                    stop=(fo == FT // FS - 1 and k == FS - 1))
                    for m in range(8):
                        ot = opool.tile([128, 512], mybir.dt.float32)
                        nc.any.tensor_copy(ot[:], pts[m][:])
                        nc.sync.dma_start(
                            out.ap()[m * 128:(m + 1) * 128,
                                     n * 512:(n + 1) * 512], ot[:])
    nc.compile()
    return nc


def dev_ffn(A, WK, WV):
    """relu(A @ WK)^2 @ WV for A [8192, C] -> [8192, C] f32."""
    key = 'ffn'
    if key not in _CACHE:
        _CACHE[key] = _build_ffn()
        _RUNNERS[key] = _make_runner(_CACHE[key], 0, 0,
                                     in_names_exp=["at", "wk", "wv"])
    run, put_weight = _RUNNERS[key]
    for wkey, Wm in (("k", WK), ("v", WV)):
        ck = (key, id(Wm))
        if ck not in _BCACHE:
            _BCACHE[ck] = (Wm, put_weight(
                np.ascontiguousarray(Wm.astype(ml_dtypes.bfloat16))))
    wk_dev = _BCACHE[(key, id(WK))][1]
    wv_dev = _BCACHE[(key, id(WV))][1]
    abf = np.ascontiguousarray(A.T.astype(ml_dtypes.bfloat16))
    at_concat = np.concatenate(
        [abf[:, i * ROWS:(i + 1) * ROWS] for i in range(NCORES)], 0)
    out = run(at_concat, wk_dev, wv_dev)
    return np.ascontiguousarray(out.reshape(NCORES * ROWS, C))


def dev_mm(A, Bm):
    """A [8192, K] @ Bm [K, N] -> [8192, N] f32, bf16 compute on 8 cores."""
    import time as _time
    t0 = _time.time()
    K, N = Bm.shape
    key = (K, N)
    if key not in _CACHE:
        _CACHE[key] = _build_mm(K, N)
        _RUNNERS[key] = _make_runner(_CACHE[key], K, N)
    run, put_weight = _RUNNERS[key]
    t1 = _time.time()
    bkey = id(Bm)
    if bkey not in _BCACHE:
        # hold a reference to Bm so its id stays valid for the cache lifetime
        bbf = np.ascontiguousarray(Bm.astype(ml_dtypes.bfloat16))
        _BCACHE[bkey] = (Bm, put_weight(bbf))
    bm_dev = _BCACHE[bkey][1]
    abf = np.ascontiguousarray(A.T.astype(ml_dtypes.bfloat16))  # [K, 8192]
    # per-core shard c takes columns [c*ROWS:(c+1)*ROWS]; concat on axis 0
    at_concat = np.concatenate(
        [abf[:, i * ROWS:(i + 1) * ROWS] for i in range(NCORES)], 0)
    t2 = _time.time()
    out = run(at_concat, bm_dev)  # [8*ROWS, N]
    t3 = _time.time()
    out = np.ascontiguousarray(out.reshape(NCORES * ROWS, N))
    if _TIMING:
        print(f"  dev_mm K={K} N={N}: build {t1-t0:.2f} prep {t2-t1:.2f} "
              f"run {t3-t2:.2f} post {_time.time()-t3:.2f}")
    return out


def _layernorm(y):
    mu = y.mean(-1, keepdims=True)
    var = y.var(-1, keepdims=True)
    return (y - mu) / np.sqrt(var + 1e-5)


def _time_shift(y):
    out = np.zeros_like(y)
    out[:, 1:] = y[:, :-1]
    return out


def _sig(z):
    return 1.0 / (1.0 + np.exp(-z))


_WKV = {}


def _wkv7(r, w, k, v, a, b):
    import jax
    import jax.numpy as jnp

    if 'fn' not in _WKV:
        def wkv7_jax(r, w, k, v, a, b):
            Bz, Tt, Cc = r.shape
            def bh(t):
                return t.astype(jnp.bfloat16).astype(jnp.float32).reshape(Bz, Tt, H, HS)
            r, k, v, a, b = map(bh, (r, k, v, a, b))
            wdec = jnp.exp(-jnp.exp(w.astype(jnp.bfloat16).astype(jnp.float32))).reshape(Bz, Tt, H, HS)
            def step(S, inp):
                rt, wt, kt, vt, at, bt = inp
                Sa = jnp.einsum('bhij,bhj->bhi', S, at)
                S = S * wt[:, :, None, :] + Sa[..., None] * bt[:, :, None, :] + vt[..., None] * kt[:, :, None, :]
                return S, jnp.einsum('bhij,bhj->bhi', S, rt)
            S0 = jnp.zeros((Bz, H, HS, HS), jnp.float32)
            seq = tuple(jnp.moveaxis(t, 1, 0) for t in (r, wdec, k, v, a, b))
            _, out = jax.lax.scan(step, S0, seq)
            return jnp.moveaxis(out, 0, 1).reshape(Bz, Tt, Cc)
        cpu = jax.devices('cpu')[0]
        _WKV['fn'] = jax.jit(wkv7_jax, device=cpu)
        _WKV['cpu'] = cpu
    with jax.default_device(_WKV['cpu']):
        out = _WKV['fn'](r, w, k, v, a, b)
    return np.asarray(out)


def kernel(x, params):
    x = np.asarray(x, np.float32)
    p = {k: np.asarray(v, np.float32) for k, v in params.items()}

    xf = x.reshape(B * T, C)

    # ---- tmix ----
    xln3 = _layernorm(x)
    xx = (_time_shift(xln3) - xln3)
    xxx = xln3 + xx * p['maa_x']
    m = np.tanh((xxx.reshape(B * T, C) @ p['maa_w1'])).reshape(B, T, 4, 32)
    mrg = (m[:, :, 0].reshape(B * T, 32) @ p['maa_w2'][0]).reshape(B, T, C)
    mwa = (m[:, :, 1].reshape(B * T, 32) @ p['maa_w2'][1]).reshape(B, T, C)
    mk_ = (m[:, :, 2].reshape(B * T, 32) @ p['maa_w2'][2]).reshape(B, T, C)
    mv = (m[:, :, 3].reshape(B * T, 32) @ p['maa_w2'][3]).reshape(B, T, C)
    xrg = (xln3 + xx * (p['maa_rg'] + mrg)).reshape(B * T, C)
    xwa = (xln3 + xx * (p['maa_wa'] + mwa)).reshape(B * T, C)
    xk = (xln3 + xx * (p['maa_k'] + mk_)).reshape(B * T, C)
    xv = (xln3 + xx * (p['maa_v'] + mv)).reshape(B * T, C)

    r = dev_mm(xrg, p['W_r'])
    k = dev_mm(xk, p['W_k'])
    v = dev_mm(xv, p['W_v'])

    w = -np.logaddexp(0.0, -(p['time_decay'] + np.tanh(xwa @ p['dec_w1']) @ p['dec_w2'])) - 0.5
    g = np.tanh(xrg @ p['gate_w1']) @ p['gate_w2']
    kk = k + np.tanh(xk @ p['kkk_w1']) @ p['kkk_w2']
    kkh = kk.reshape(B * T, H, HS)
    kk = (kkh / np.maximum(np.linalg.norm(kkh, axis=-1, keepdims=True), 1e-12)).reshape(B * T, C)
    a = _sig(p['time_aaaaa'] + xwa @ p['aaa_w1'] @ p['aaa_w2'])
    ma = _sig(xwa @ p['ma_w1'] @ p['ma_w2'])
    k = k * ma + k * a * (1.0 - ma)
    mk2 = _sig(xk @ p['mk_w1'] @ p['mk_w2'])
    k = k * np.exp(np.minimum(w * mk2, 0.0))

    sh = (B, T, C)
    o = _wkv7(r.reshape(sh), w.reshape(sh), k.reshape(sh), v.reshape(sh),
              (-kk).reshape(sh), (kk * a).reshape(sh)).reshape(B * T, C)

    og = o.reshape(B * T, H, HS)
    mu = og.mean(-1, keepdims=True)
    var = og.var(-1, keepdims=True)
    o = ((og - mu) / np.sqrt(var + 64e-5)).reshape(B * T, C)
    rk = (r.reshape(B * T, H, HS) * k.reshape(B * T, H, HS) * p['time_faaaa']).sum(-1, keepdims=True)
    o = o + (rk * v.reshape(B * T, H, HS)).reshape(B * T, C)

    h = xf + dev_mm(o * g, p['W_o'])

    # ---- cmix ----
    h3 = h.reshape(B, T, C)
    hln = _layernorm(h3)
    xx2 = _time_shift(hln) - hln
    kf = (hln + xx2 * p['cm_maa_k']).reshape(B * T, C)
    out = h + dev_ffn(kf, p['cm_key'], p['cm_val'])
    return out.reshape(B, T, C).astype(np.float32)
